# revision 5
# baseline (speedup 1.0000x reference)
"""BatchNorm over batch axis (N=131072, D=512) on 8 trn2 NeuronCores.

v5: single-HBM-pass design. From the v2/v3/v4 traces + microbench:
 - Pass 1 (DMA-bound ~100 us): stream X f32 once; DVE casts each tile
   to the fp16 SBUF cache (1.23 us) and accumulates fp16 x-sums
   (1.22 us); ACT squares f32->fp16 (2.0 us); PE does 4 ones-matmuls
   per tile on sq only (~0.35 us each, stays HAM-warm). stream bufs=4
   (3 bufs starved the DMA queue in v3).
 - Stats all-reduce via fp16 AllGather (4.6 us floor vs ~10+ for
   AllReduce) + one 8-partition ones-matmul to reduce the gathered
   shards; scalar_tensor_tensor fuses the 1/N normalizations.
 - Pass 2: Y = x*scale + bias entirely on DVE fp16 2x tensor_tensor
   (~1.22 us/op, 64 ops), fp16 Y stores (host upcasts). No GPSIMD
   (v3 measured 5 us + 0.9 us drain per op, serial -> 70 us tail).
"""

import numpy as np
from contextlib import ExitStack

import concourse.bass as bass
import concourse.bacc as bacc
import concourse.tile as tile
from concourse import mybir
from concourse.bass_utils import run_bass_kernel_spmd

N, D = 131072, 512
NCORES = 8
NP = N // NCORES  # rows per core
P = 128           # SBUF partitions
RB = 4            # 128-row blocks per tile -> 512 rows, 1 MiB f32 per DMA
F32 = mybir.dt.float32
F16 = mybir.dt.float16

_cache = {}


def flat(ap):
    return ap.rearrange("p b d -> p (b d)")


def _build(np_rows=NP, n_total=N):
    rows_per_tile = P * RB
    nt = np_rows // rows_per_tile
    assert nt * rows_per_tile == np_rows

    nc = bacc.Bacc(num_devices=NCORES)
    X = nc.declare_dram_parameter("X", [np_rows, D], F32, isOutput=False)
    gamma = nc.declare_dram_parameter("gamma", [1, D], F32, isOutput=False)
    beta = nc.declare_dram_parameter("beta", [1, D], F32, isOutput=False)
    Y = nc.declare_dram_parameter("Y", [np_rows, D], F16, isOutput=True)
    cc_in = nc.dram_tensor("cc_in", [1, 2, D], F16)
    cc_out = nc.dram_tensor("cc_out", [NCORES, 2, D], F16, addr_space="Shared")
    stats_dram = nc.dram_tensor("stats_dram", [1, 2, D], F32)
    bar_in = nc.dram_tensor("bar_in", [1, 8], F32)
    bar_out = nc.dram_tensor("bar_out", [1, 8], F32, addr_space="Shared")
    barg_out = nc.dram_tensor("barg_out", [NCORES, 8], F32, addr_space="Shared")

    Xv = X[:].rearrange("(t p b) d -> t p b d", p=P, b=RB)
    Yv = Y[:].rearrange("(t p b) d -> t p b d", p=P, b=RB)

    with tile.TileContext(nc) as tc, ExitStack() as ctx:
        stream = ctx.enter_context(tc.tile_pool(name="stream", bufs=4))
        sqpool = ctx.enter_context(tc.tile_pool(name="sq", bufs=2))
        cpool = ctx.enter_context(tc.tile_pool(name="cache", bufs=nt))
        singles = ctx.enter_context(tc.tile_pool(name="singles", bufs=1))
        psum = ctx.enter_context(tc.tile_pool(name="psum", bufs=1, space="PSUM"))

        # early rendezvous barrier: absorbs the ~65us first-collective warmup
        # and inter-core kernel-start skew while pass-1 streaming runs; the
        # dummy AllGather also pre-warms the AG SPAD path for the real one
        barz = singles.tile([1, 8], F32)
        nc.vector.memset(barz[:], 0.0)
        nc.gpsimd.dma_start(out=bar_in[:], in_=barz[:])
        nc.gpsimd.collective_compute(
            "AllReduce",
            mybir.AluOpType.add,
            replica_groups=[list(range(NCORES))],
            ins=[bar_in[:].opt()],
            outs=[bar_out[:].opt()],
        )
        nc.gpsimd.collective_compute(
            "AllGather",
            mybir.AluOpType.bypass,
            replica_groups=[list(range(NCORES))],
            ins=[bar_in[:].opt()],
            outs=[barg_out[:].opt()],
        )

        ones = singles.tile([P, 1], F16)
        nc.vector.memset(ones[:], 1.0)
        # pre-warm the ACT sqrt table and DVE reciprocal ucode off the
        # critical path
        ones_f = singles.tile([P, 1], F32)
        nc.vector.memset(ones_f[:], 1.0)
        warm = singles.tile([P, 2], F32)
        nc.scalar.sqrt(warm[:, 0:1], ones_f[:])
        nc.vector.reciprocal(warm[:, 1:2], ones_f[:])

        # gamma/beta broadcast early; rides under pass-1 streaming
        gb = singles.tile([P, 2, D], F32)
        nc.scalar.dma_start(out=gb[:, 0, :], in_=gamma[:].to_broadcast((P, D)))
        nc.scalar.dma_start(out=gb[:, 1, :], in_=beta[:].to_broadcast((P, D)))

        ps_x = psum.tile([1, D], F32)
        ps_x2 = psum.tile([1, D], F32)

        acc = singles.tile([P, RB, D], F16)  # fp16 x-sums, folded in place
        nc.vector.memset(acc[:], 0.0)

        # --- pass 1: stream X once; fp16 cache + per-core partial sums ---
        cache_tiles = []
        for t in range(nt):
            xt = stream.tile([P, RB, D], F32, tag="xt")
            lq = nc.sync if t % 2 == 0 else nc.scalar
            lq.dma_start(out=xt[:], in_=Xv[t])
            ct = cpool.tile([P, RB, D], F16, tag="cache", name=f"ct{t}")
            nc.vector.tensor_copy(flat(ct[:]), flat(xt[:]))
            nc.vector.tensor_add(flat(acc[:]), flat(acc[:]), flat(ct[:]))
            sq = sqpool.tile([P, RB, D], F16, tag="sq")
            nc.scalar.square(flat(sq[:]), flat(xt[:]))
            for b in range(RB):
                nc.tensor.matmul(
                    ps_x2[:],
                    lhsT=ones[:],
                    rhs=sq[:, b, :],
                    start=(t == 0 and b == 0),
                    stop=(t == nt - 1 and b == RB - 1),
                )
            cache_tiles.append(ct)

        # fold fp16 x-sums in place, then one cross-partition ones-matmul
        nc.vector.tensor_add(
            flat(acc[:, 0:2, :]), flat(acc[:, 0:2, :]), flat(acc[:, 2:4, :])
        )
        nc.vector.tensor_add(acc[:, 0, :], acc[:, 0, :], acc[:, 1, :])
        nc.tensor.matmul(
            ps_x[:], lhsT=ones[:], rhs=acc[:, 0, :], start=True, stop=True
        )

        stage = singles.tile([1, 2, D], F16)
        nc.scalar.copy(stage[:, 0, :], ps_x[:])
        nc.scalar.copy(stage[:, 1, :], ps_x2[:])

        # --- all-gather the per-core fp16 raw sums, reduce via 8-row matmul
        nc.gpsimd.dma_start(out=cc_in[:], in_=stage[:])
        nc.gpsimd.collective_compute(
            "AllGather",
            mybir.AluOpType.bypass,
            replica_groups=[list(range(NCORES))],
            ins=[cc_in[:].opt()],
            outs=[cc_out[:].opt()],
        )
        g = singles.tile([NCORES, 2, D], F16)
        nc.scalar.dma_start(out=g[:], in_=cc_out[:])
        ones8 = singles.tile([NCORES, 1], F16)
        nc.vector.memset(ones8[:], 1.0)
        pg = psum.tile([1, 2, D], F32)
        nc.tensor.matmul(pg[:, 0, :], lhsT=ones8[:], rhs=g[:, 0, :],
                         start=True, stop=True)
        nc.tensor.matmul(pg[:, 1, :], lhsT=ones8[:], rhs=g[:, 1, :],
                         start=True, stop=True)
        stage3 = singles.tile([1, 2, D], F32)
        nc.scalar.copy(stage3[:], pg[:])

        # --- stats -> scale/bias, replicated on all partitions ---
        # sums stay RAW (no 1/N pass); scalar_tensor_tensor fuses the 1/N:
        #   msq  = square(sx/N)              (ACT, scale=1/N)
        #   var  = (m2 * 1/N) - msq          (DVE stt)
        #   sd   = sqrt(var)                 (ACT)
        #   inv  = 1/sd                      (DVE recip)
        #   scale= gamma*inv                 (DVE, in place)
        #   bias = beta - (sx*scale)/N       (DVE mul + stt)
        nc.scalar.dma_start(out=stats_dram[:], in_=stage3[:])
        sums = singles.tile([P, 2, D], F32)
        nc.scalar.dma_start(out=sums[:], in_=stats_dram[:].to_broadcast((P, 2, D)))
        scr = singles.tile([P, 2, D], F32)
        inv_n = 1.0 / n_total
        sx, m2 = sums[:, 0, :], sums[:, 1, :]
        msq, tmp = scr[:, 0, :], scr[:, 1, :]
        nc.scalar.activation(
            msq, sx, mybir.ActivationFunctionType.Square, scale=inv_n
        )
        SB = singles.tile([P, 2, D], F32)  # [:,0]=scale  [:,1]=bias
        var = m2  # overwritten in place
        nc.vector.scalar_tensor_tensor(
            out=var, in0=m2, scalar=inv_n, in1=msq,
            op0=mybir.AluOpType.mult, op1=mybir.AluOpType.subtract,
        )
        nc.scalar.sqrt(msq, var)
        nc.vector.reciprocal_approx_accurate(out=SB[:, 0, :], in_=msq, scratch=tmp)
        nc.vector.tensor_mul(SB[:, 0, :], SB[:, 0, :], gb[:, 0, :])
        nc.vector.tensor_mul(tmp, sx, SB[:, 0, :])
        nc.vector.scalar_tensor_tensor(
            out=SB[:, 1, :], in0=tmp, scalar=-inv_n, in1=gb[:, 1, :],
            op0=mybir.AluOpType.mult, op1=mybir.AluOpType.add,
        )

        # fp16 scale/bias replicated RB-fold along free dim -> plain step-1
        # APs in pass 2
        SBF = singles.tile([P, 2, RB, D], F16)
        nc.vector.tensor_copy(SBF[:, :, 0, :], SB[:])
        nc.vector.tensor_copy(SBF[:, :, 1, :], SBF[:, :, 0, :])
        nc.vector.tensor_copy(SBF[:, :, 2:4, :], SBF[:, :, 0:2, :])
        scf = flat(SBF[:, 0])
        bif = flat(SBF[:, 1])

        # --- pass 2: Y = x*scale + bias, in-place in the fp16 cache ---
        for t in range(nt):
            ct = cache_tiles[t]
            nc.vector.tensor_mul(flat(ct[:]), flat(ct[:]), scf)
            nc.vector.tensor_add(flat(ct[:]), flat(ct[:]), bif)
            nc.scalar.dma_start(out=Yv[t], in_=ct[:])

    nc.compile()
    return nc


def _get_nc(np_rows=NP, n_total=N):
    key = (np_rows, n_total)
    if key not in _cache:
        _cache[key] = _build(np_rows, n_total)
    return _cache[key]


def _run(X, gamma, beta, trace=False):
    X = np.ascontiguousarray(np.asarray(X, dtype=np.float32))
    g = np.ascontiguousarray(np.asarray(gamma, dtype=np.float32).reshape(1, D))
    b = np.ascontiguousarray(np.asarray(beta, dtype=np.float32).reshape(1, D))
    rows = X.shape[0]
    per = rows // NCORES
    nc = _get_nc(per, rows)
    in_maps = [
        {"X": X[i * per:(i + 1) * per], "gamma": g, "beta": b}
        for i in range(NCORES)
    ]
    res = run_bass_kernel_spmd(nc, in_maps, list(range(NCORES)), trace=trace)
    out = np.concatenate(
        [np.asarray(res.results[i]["Y"], dtype=np.float32) for i in range(NCORES)],
        axis=0,
    )
    return out, res


def kernel(X, gamma, beta):
    out, _ = _run(X, gamma, beta, trace=False)
    return out
